# revision 55
# baseline (speedup 1.0000x reference)
"""Trainium2 Bass kernel for per-edge dot products (GNN DotPredictor).

out[e] = sum(h[src[e]] * h[dst[e]]); 800k edges, h [50k, 64] f32, 8 cores.

Design (v8, ~3.4x over the v2 baseline):
  - Edges sharded 8 ways; h replicated. Per-edge rows fetched from HBM with
    the Q7 `dma_gather` path. The serial resource is Q7 descriptor
    generation (~7.5ns/descriptor on one cpu pair); this kernel runs
    num_swdge_queues=4 so four cpu pairs (queue_num 0-3) generate
    concurrently. Gathers are split into <=4096-descriptor pieces and
    greedy-packed onto the least-loaded queue (dedicated DMA-completion
    semaphore per gather: SWDGE sems are queue-locked, and the 16 unordered
    per-DMA-engine increments make shared-sem thresholds racy).
  - Descriptor minimization: edges sorted by (range-group, src); equal-src
    runs decomposed into K-edge units (K in {8,4,2,1}); ONE 256B src
    descriptor per unit serves K edges (DVE broadcasts the row). dst side
    is one 256B descriptor per edge (~104k + ~27k descriptors/core,
    structurally near-minimal: any single edge order leaves one side
    random).
  - int16 gather indices => 4-way range bucketing (src>=32768, dst>=32768)
    with per-range base pointers; host permutes edges, unpermutes results.
  - Pipeline: 6 hu/hv buffer sets, 8192-edge chunks; DVE multiplies hu
    (step-0 broadcast AP) into the hv tile in place and segment-reduces the
    64-wide feature dim; num_idxs registers hoisted; idx upload split so
    the first chunks start during the Q7 library load; output stored in
    two waves.
"""

import os
from contextlib import ExitStack

import numpy as np

import concourse.bacc as bacc
import concourse.mybir as mybir
from concourse import library_config
from concourse.bass import AP
from concourse._compat import get_trn_type
from concourse.bass_utils import run_bass_kernel_spmd

N_NODES = 50000
NPAD = 50008  # h padded so K-row reads past the last node stay in bounds
D = 64
P = 128
N_CORES = 8
SPLIT = 32768

G_MAP = {8: 1024, 4: 2048, 2: 4096, 1: 4096}  # units per chunk (<=8192 edges)

TRACE = False
LAST_RESULT = None



def _ensure_ntff_hook():
    """bass_utils' trace path imports antenv.axon_hooks, which this image's
    antenv package lacks. Recreate it from the boot helper so trace=True
    works; harmless no-op if the real module exists."""
    import sys
    import types

    try:
        import antenv.axon_hooks  # noqa: F401

        return
    except ImportError:
        pass
    try:
        import antenv
        from trn_agent_boot.trn_boot import _ntff_profile_via_ctypes

        hook = _ntff_profile_via_ctypes("/opt/axon/libaxon_pjrt.so")
        m = types.ModuleType("antenv.axon_hooks")
        m.get_axon_ntff_profile_hook = lambda: hook
        m.set_axon_ntff_profile_hook = lambda h: None
        sys.modules["antenv.axon_hooks"] = m
        antenv.axon_hooks = m
    except Exception:
        pass


def _wrap_idx(vals):
    """int16 index array [Npc] -> the [128, Npc/16] SBUF layout dma_gather
    expects (idx i at partition i%16, column i//16, replicated over the 8
    groups of 16 partitions)."""
    w = vals.reshape(-1, 16).T  # [16, Npc/16]
    return np.ascontiguousarray(np.tile(w, (8, 1)))  # [128, Npc/16]


def _host_prep(src, dst):
    """Sort by (range-group, src); decompose equal-src runs into K-units.

    Returns (schedule, seqs, sidx_per_core, didx_per_core, u_total, e_total):
      schedule: list of (K, s_hi, d_hi, u_off, e_off, n_units), same all cores
      seqs: [N_CORES, e_total] global edge id per output position (-1 pad)
    """
    E = src.shape[0]
    g = (src >= SPLIT).astype(np.int8) * 2 + (dst >= SPLIT).astype(np.int8)
    order0 = np.lexsort((src, g))
    sg, ss, sd = g[order0], src[order0], dst[order0]

    new = np.ones(E, bool)
    new[1:] = (sg[1:] != sg[:-1]) | (ss[1:] != ss[:-1])
    run_start = np.flatnonzero(new)
    d = np.diff(np.append(run_start, E))
    run_id = np.cumsum(new) - 1
    r = np.arange(E) - run_start[run_id]
    dd = d[run_id]
    n8 = (dd // 8) * 8
    n4 = n8 + (((dd - n8) // 4) * 4)
    n2 = n4 + (((dd - n4) // 2) * 2)
    K_e = np.where(r < n8, 8, np.where(r < n4, 4, np.where(r < n2, 2, 1)))
    m_e = np.where(
        K_e == 8, r % 8,
        np.where(K_e == 4, (r - n8) % 4, np.where(K_e == 2, (r - n4) % 2, 0)),
    )
    first = m_e == 0

    pad_units = N_CORES * P
    schedule = []
    sidx_parts = [[] for _ in range(N_CORES)]
    didx_parts = [[] for _ in range(N_CORES)]
    seq_parts = [[] for _ in range(N_CORES)]
    u_off = 0
    e_off = 0
    for K in (8, 4, 2, 1):
        for gg in range(4):
            starts = np.flatnonzero(first & (K_e == K) & (sg == gg))
            if starts.size == 0:
                continue
            Upad = -(-starts.size // pad_units) * pad_units
            buf = np.full(Upad, -1, dtype=np.int64)
            buf[: starts.size] = starts
            U = Upad // N_CORES  # per-core units, multiple of 128
            s_hi, d_hi = gg >= 2, gg % 2 == 1
            for c in range(N_CORES):
                uc = buf[c * U : (c + 1) * U]
                valid = uc >= 0
                sv = np.zeros(U, np.int64)
                sv[valid] = ss[uc[valid]] - (SPLIT if s_hi else 0)
                sidx_parts[c].append(sv.astype(np.int16))
                dvals = np.zeros(U * K, np.int64)
                ids = np.full(U * K, -1, np.int64)
                uu = np.arange(U)
                for m in range(K):
                    pos = (K * (uu // P) + m) * P + uu % P
                    dvals[pos[valid]] = sd[uc[valid] + m] - (
                        SPLIT if d_hi else 0
                    )
                    ids[pos[valid]] = order0[uc[valid] + m]
                didx_parts[c].append(dvals.astype(np.int16))
                seq_parts[c].append(ids)
            # chunks
            o, rem = 0, U
            Gn = G_MAP[K]
            while rem > 0:
                n = min(Gn, rem)
                schedule.append((K, s_hi, d_hi, u_off + o, e_off + o * K, n))
                o += n
                rem -= n
            u_off += U
            e_off += U * K

    seqs = np.stack([np.concatenate(p) for p in seq_parts])
    sidx = [np.concatenate(p) for p in sidx_parts]
    didx = [np.concatenate(p) for p in didx_parts]
    return schedule, seqs, sidx, didx, u_off, e_off


NQ = 4  # SWDGE queues (Q7 cpu pairs generating descriptors in parallel)
NB = 6  # hu/hv buffer sets in rotation
HVC = 4096  # hv tile cols (8192 edges)
HUC = 2048  # hu tile cols (4096 units)


def _build_nc(schedule, u_total, e_total):
    SCOLS = u_total // 16
    DCOLS = e_total // 16
    TILES = e_total // P

    nc = bacc.Bacc(
        get_trn_type() or "TRN2",
        debug=False,
        dynamic_dma_scratch_size=32768,
        num_swdge_queues=NQ,
    )
    h = nc.dram_tensor("h", [NPAD, D], mybir.dt.float32, kind="ExternalInput")
    sidx = nc.dram_tensor("sidx", [P, SCOLS], mybir.dt.int16, kind="ExternalInput")
    didx = nc.dram_tensor("didx", [P, DCOLS], mybir.dt.int16, kind="ExternalInput")
    out = nc.dram_tensor("out", [P, TILES], mybir.dt.float32, kind="ExternalOutput")

    h_ap = h[:]
    # per-edge dst bases (rows of 64)
    hd_lo = h[0:SPLIT, :]
    hd_hi = h[SPLIT:NPAD, :]
    nch = len(schedule)

    with ExitStack() as stack:
        ent = stack.enter_context
        hu = [ent(nc.sbuf_tensor(f"hu{i}", [P, HUC], mybir.dt.float32)) for i in range(NB)]
        hv = [ent(nc.sbuf_tensor(f"hv{i}", [P, HVC], mybir.dt.float32)) for i in range(NB)]
        sidx_sb = ent(nc.sbuf_tensor("sidx_sb", [P, SCOLS], mybir.dt.int16))
        didx_sb = ent(nc.sbuf_tensor("didx_sb", [P, DCOLS], mybir.dt.int16))
        outb = ent(nc.sbuf_tensor("outb", [P, TILES], mybir.dt.float32))
        io = ent(nc.semaphore("io"))
        iof = ent(nc.semaphore("iof"))
        io2 = ent(nc.semaphore("io2"))
        # One dedicated DMA-completion sem per gather: SWDGE sems are locked
        # to a single queue, and 16 unordered DMA-engine increments make any
        # shared-sem intermediate threshold racy. Each chunk has up to 3
        # gathers: src, dst half 0, dst half 1.
        # Balance descriptor-generation load across the 4 Q7 cpu pairs:
        # greedy bin-packing (emission order, least-loaded queue first).
        # Gathers are split into <=PIECE-desc pieces for finer balance.
        PIECE = 4096

        def split_sizes(total):
            pc = -(-total // PIECE)
            base = total // pc // 128 * 128
            sizes = [base] * pc
            sizes[-1] = total - base * (pc - 1)
            return sizes

        qload = [0] * NQ

        def pick_queue(load):
            q = min(range(NQ), key=lambda i: (qload[i], i))
            qload[q] += load
            return q

        # qplan[c] = (src_pieces, dst_pieces): lists of (offset, size, queue)
        def dst_sizes(ne):
            if ne < 2048:
                return split_sizes(ne)
            return [ne // 2 // 128 * 128, ne - ne // 2 // 128 * 128]

        qplan = []
        for c, (K, s_hi, d_hi, uo, eo, n) in enumerate(schedule):
            sp = []
            o = 0
            for sz in split_sizes(n):
                sp.append((o, sz, pick_queue(sz)))
                o += sz
            dp = []
            o = 0
            for sz in dst_sizes(n * K):
                dp.append((o, sz, pick_queue(sz)))
                o += sz
            qplan.append((sp, dp))

        gsems = [
            [ent(nc.semaphore(f"g{c}_{j}")) for j in range(len(qplan[c][0]) + len(qplan[c][1]))]
            for c in range(nch)
        ]
        vsem = [ent(nc.semaphore(f"v{i}")) for i in range(NB)]
        mr = ent(nc.semaphore("mr"))

        def hu_ap(b, t_u, off=0):
            base = hu[b][:]
            return AP(base.tensor, off * D, [[HUC, P], [D, t_u], [1, D]])

        def hu_bcast(b, t_u, K):
            base = hu[b][:]
            return AP(base.tensor, 0, [[HUC, P], [D, t_u], [0, K], [1, D]])

        def hv_ap(b, t_e, off=0):
            base = hv[b][:]
            return AP(base.tensor, off * D, [[HVC, P], [D, t_e], [1, D]])

        def hv_4d(b, t_u, K):
            base = hv[b][:]
            return AP(base.tensor, 0, [[HVC, P], [D * K, t_u], [D, K], [1, D]])

        def hsrc_ap(s_hi):
            if s_hi:
                return AP(h_ap.tensor, SPLIT * D, [[D, 17232], [1, D]])
            return AP(h_ap.tensor, 0, [[D, SPLIT], [1, D]])

        # first-wave idx slice: small, so it lands before the Q7 library
        # load finishes and the first gathers start immediately
        nw = min(2, nch)
        u_first = schedule[nw - 1][3] + schedule[nw - 1][5] if nw else 0
        e_first = schedule[nw - 1][4] + schedule[nw - 1][5] * schedule[nw - 1][0]
        sf_cols = u_first // 16
        df_cols = e_first // 16

        with nc.Block() as block:

            @block.sync
            def _(sync):
                sync.dma_start(sidx_sb[:, :sf_cols], sidx[:, :sf_cols]).then_inc(iof, 16)
                sync.dma_start(didx_sb[:, :df_cols], didx[:, :df_cols]).then_inc(iof, 16)
                if sf_cols < SCOLS:
                    sync.dma_start(sidx_sb[:, sf_cols:], sidx[:, sf_cols:]).then_inc(io, 16)
                if df_cols < DCOLS:
                    sync.dma_start(didx_sb[:, df_cols:], didx[:, df_cols:]).then_inc(io, 16)
                # store the first half of outb as soon as the first mid chunks
                # are reduced; store the rest when everything is done.
                mid = nch // 2
                mtile = schedule[mid][4] // P if mid else 0
                if mtile:
                    for i in range(NB):
                        cnt = len([c for c in range(mid) if c % NB == i])
                        if cnt:
                            sync.wait_ge(vsem[i], cnt)
                    sync.dma_start(out[:, :mtile], outb[:, :mtile]).then_inc(io2, 16)
                for i in range(NB):
                    cnt = len([c for c in range(nch) if c % NB == i])
                    if cnt:
                        sync.wait_ge(vsem[i], cnt)
                sync.dma_start(out[:, mtile:], outb[:, mtile:]).then_inc(io2, 16)
                sync.wait_ge(io2, 32 if mtile else 16)

            io_rest = 16 * int(sf_cols < SCOLS) + 16 * int(df_cols < DCOLS)

            @block.gpsimd
            def _(gp):
                gp.load_library(library_config.mlp)
                # hoist num_idxs registers: to_reg emits a Pool MOVE per call
                # (~0.4us sequencer each); values repeat heavily across chunks.
                nvals = set()
                for sp, dp in qplan:
                    nvals.update(sz for _, sz, _ in sp)
                    nvals.update(sz for _, sz, _ in dp)
                nreg = {v: gp.to_reg(v) for v in sorted(nvals)}
                gp.wait_ge(iof, 32)
                for c, (K, s_hi, d_hi, uo, eo, n) in enumerate(schedule):
                    b = c % NB
                    if c == nw and io_rest:
                        gp.wait_ge(io, io_rest)
                    if c >= NB:
                        gp.wait_ge(vsem[b], c // NB)
                    sp, dp = qplan[c]
                    hd = hd_hi if d_hi else hd_lo
                    for j, (o, sz, q) in enumerate(sp):
                        gp.dma_gather(
                            hu_ap(b, sz // P, off=o // P),
                            hsrc_ap(s_hi),
                            sidx_sb[:, (uo + o) // 16 : (uo + o + sz) // 16],
                            sz,
                            nreg[sz],
                            D,
                            single_packet=False,
                            queue_num=q,
                        ).then_inc(gsems[c][j], 16)
                    for j, (o, sz, q) in enumerate(dp):
                        gp.dma_gather(
                            hv_ap(b, sz // P, off=o // P),
                            hd,
                            didx_sb[:, (eo + o) // 16 : (eo + o + sz) // 16],
                            sz,
                            nreg[sz],
                            D,
                            single_packet=False,
                            queue_num=q,
                        ).then_inc(gsems[c][len(sp) + j], 16)

            @block.vector
            def _(ve):
                for c, (K, s_hi, d_hi, uo, eo, n) in enumerate(schedule):
                    b = c % NB
                    for s in gsems[c]:
                        ve.wait_ge(s, 16)
                    t_u = n // P
                    t_e = t_u * K
                    if K == 1:
                        prod_in1 = hu_ap(b, t_u)
                        prod = hv_ap(b, t_e)
                    else:
                        prod_in1 = hu_bcast(b, t_u, K)
                        prod = hv_4d(b, t_u, K)
                    ve.tensor_tensor(
                        out=prod, in0=prod, in1=prod_in1,
                        op=mybir.AluOpType.mult,
                    ).then_inc(mr, 1)
                    ve.wait_ge(mr, c + 1)
                    ve.tensor_reduce(
                        out=outb[:, eo // P : eo // P + t_e],
                        in_=prod,
                        axis=mybir.AxisListType.X,
                        op=mybir.AluOpType.add,
                    ).then_inc(vsem[b], 1)

    nc.compile()
    return nc


def kernel(h, src, dst):
    global LAST_RESULT
    h = np.asarray(h, dtype=np.float32)
    hp = np.zeros((NPAD, D), np.float32)
    hp[:N_NODES] = h
    src = np.asarray(src).astype(np.int64)
    dst = np.asarray(dst).astype(np.int64)
    E = src.shape[0]

    schedule, seqs, sidx, didx, u_total, e_total = _host_prep(src, dst)
    in_maps = [
        {"h": hp, "sidx": _wrap_idx(sidx[c]), "didx": _wrap_idx(didx[c])}
        for c in range(N_CORES)
    ]
    nc = _build_nc(schedule, u_total, e_total)

    if TRACE or os.environ.get("BASS_TRACE"):
        _ensure_ntff_hook()
    res = run_bass_kernel_spmd(nc, in_maps, core_ids=list(range(N_CORES)), trace=TRACE)
    LAST_RESULT = res

    out = np.empty(E, np.float32)
    for c in range(N_CORES):
        dots = res.results[c]["out"].T.reshape(-1)
        seq = seqs[c]
        valid = seq >= 0
        out[seq[valid]] = dots[valid]
    return out

